# revision 23
# baseline (speedup 1.0000x reference)
"""Triplane embedding-lookup + MLP kernel for Trainium2 (8 NeuronCores).

Strategy (v2, dma_gather):
  - Host: patch table PT[3*512*512, 128] bf16; row (pl,y,x) = 4 bilinear-corner
    pixel vectors (32ch each). Points are bucketed by (y-band, z-band) into
    8x8=64 groups of 2048 slots per core, so every group's gathers hit a
    <=32768-row band slice of PT -- addressable by dma_gather's int16 indices.
  - Device, per group: compute local cell ids (int16) + bilinear weights,
    3 dma_gather calls (2048 rows each, one SWDGE instr apiece -- ~20x less
    Pool time than per-partition indirect DMA), bf16 weighted combine on DVE,
    4-layer MLP on PE (bf16), out [1, 2048] f32 to DRAM.
  - Host: inverse-permute the per-slot outputs back to input point order.
"""

import sys

sys.path.insert(0, "/opt/trn_rl_repo")

from contextlib import ExitStack

import numpy as np

RES = 512
CELLS = RES * RES
EMB = 32
HID = 128
N = 1_000_000
NCORES = 8

BANDS = 8          # bands per axis
BROWS = RES // BANDS   # 64 rows per band
BCELLS = BROWS * RES   # 32768 cells per band slice (int16-addressable)
NGRP = BANDS * BANDS   # 64 groups per core: (y-band, z-band)
S = 2048           # point slots per group
SJ = S // 128      # 16 free-dim cols per partition (weights layout)
NIC = S // 16      # 128 idx cols per partition (idx layout)
NBT = 4            # MLP batches of 512 points per group
NP_CORE = NGRP * S  # 131072 slots per core

LAST_RESULTS = None
_BUILT = {}

# HW f32->i32 cast is rint => floor(pix) == rint(pix - 0.5), bias 255.0.
# CoreSim casts via numpy truncation => floor(pix) == trunc(pix), bias 255.5.
SIM_MODE = False


def _cast_bias() -> float:
    return 255.5 if SIM_MODE else 255.0


def _build_nc(table_dt_name: str = "bfloat16"):
    from concourse import bacc, bass, mybir, library_config
    import concourse.tile as tile
    from concourse.masks import make_identity

    dt = mybir.dt
    tdt = getattr(dt, table_dt_name)
    f32 = dt.float32
    i32 = dt.int32
    i16 = dt.int16
    bf16 = dt.bfloat16
    mult = mybir.AluOpType.mult
    add = mybir.AluOpType.add
    amax = mybir.AluOpType.max
    amin = mybir.AluOpType.min
    AF = mybir.ActivationFunctionType

    # 4 SWDGE queues => 4 Q7 core-pairs generate gather descriptors
    # concurrently (measured 3.6x on HW vs 1 queue).
    nc = bacc.Bacc("TRN2", target_bir_lowering=False, num_swdge_queues=4)

    # BCELLS guard rows on both ends: wrapped/mismatched int16 indices
    # (+-32K rows) always land inside the tensor instead of faulting.
    ptd = nc.dram_tensor("pt", [3 * CELLS + 2 * BCELLS, 128], tdt,
                         kind="ExternalInput")
    crdw = nc.dram_tensor("crdw", [NGRP, 128, 6 * SJ], f32, kind="ExternalInput")
    crdi = nc.dram_tensor("crdi", [NGRP, 128, 3 * NIC], f32, kind="ExternalInput")
    w0d = nc.dram_tensor("w0t", [EMB, HID], bf16, kind="ExternalInput")
    w1d = nc.dram_tensor("w1t", [HID, HID], bf16, kind="ExternalInput")
    w2d = nc.dram_tensor("w2t", [HID, HID], bf16, kind="ExternalInput")
    w3d = nc.dram_tensor("w3t", [HID, 1], bf16, kind="ExternalInput")
    b0d = nc.dram_tensor("b0c", [HID, 1], f32, kind="ExternalInput")
    b1d = nc.dram_tensor("b1c", [HID, 1], f32, kind="ExternalInput")
    b2d = nc.dram_tensor("b2c", [HID, 1], f32, kind="ExternalInput")
    b3d = nc.dram_tensor("b3c", [1, 1], f32, kind="ExternalInput")
    outd = nc.dram_tensor("out", [NP_CORE], f32, kind="ExternalOutput")
    outv = outd[:].unsqueeze(0)

    with tile.TileContext(nc) as tc, ExitStack() as ctx:
        nc.gpsimd.load_library(library_config.mlp)

        cpool = ctx.enter_context(tc.tile_pool(name="consts", bufs=1))

        def const_tile(shape, dtp, tag):
            return cpool.tile(shape, dtp, tag=tag, name=tag)

        w0s = const_tile([EMB, HID], bf16, "w0s")
        w1s = const_tile([HID, HID], bf16, "w1s")
        w2s = const_tile([HID, HID], bf16, "w2s")
        w3s = const_tile([HID, 1], bf16, "w3s")
        b0s = const_tile([HID, 1], f32, "b0s")
        b1s = const_tile([HID, 1], f32, "b1s")
        b2s = const_tile([HID, 1], f32, "b2s")
        b3s = const_tile([1, 1], f32, "b3s")
        ident = const_tile([128, 128], bf16, "ident")
        for s_, d_ in ((w0s, w0d), (w1s, w1d), (w2s, w2d), (w3s, w3d),
                       (b0s, b0d), (b1s, b1d), (b2s, b2d), (b3s, b3d)):
            nc.sync.dma_start(s_[:], d_[:])
        make_identity(nc, ident[:])

        work = ctx.enter_context(tc.tile_pool(name="work", bufs=2))
        gpool = ctx.enter_context(tc.tile_pool(name="gather", bufs=2))
        psum = ctx.enter_context(tc.tile_pool(name="psum", bufs=2, space="PSUM"))

        def wt(shape, dtp, tag, bufs=2):
            return work.tile(shape, dtp, tag=tag, name=tag, bufs=bufs)

        for g in range(NGRP):
            ab, bb = g // BANDS, g % BANDS

            # ---- index path (idx-wrapped layout: point i at (i%16, i//16),
            #      replicated x8 down partitions; cols [x NIC | y NIC | z NIC])
            ct = wt([128, 3 * NIC], f32, "ct", bufs=3)
            nc.sync.dma_start(ct[:], crdi[g])
            pm = wt([128, 4 * NIC], f32, "pm", bufs=3)
            cb = _cast_bias()
            nc.scalar.activation(pm[:, 0:2 * NIC], ct[:, 0:2 * NIC], AF.Copy,
                                 bias=cb, scale=255.5)              # px|py
            nc.scalar.activation(pm[:, 2 * NIC:3 * NIC], ct[:, NIC:2 * NIC],
                                 AF.Copy, bias=cb - 64.0 * ab, scale=255.5)
            nc.scalar.activation(pm[:, 3 * NIC:4 * NIC], ct[:, 2 * NIC:3 * NIC],
                                 AF.Copy, bias=cb - 64.0 * bb, scale=255.5)
            ci = wt([128, 4 * NIC], i32, "ci", bufs=3)
            nc.scalar.activation(ci[:], pm[:], AF.Copy)  # f32->i32 rint on HW
            civ = ci[:].rearrange("p (s t) -> p s t", s=4)
            # int16 out directly (values <= 32766 when host/device agree;
            # rare mismatches wrap into the table's guard rows)
            idx16 = wt([128, 3 * NIC], i16, "idx16", bufs=3)
            iv16 = idx16[:].rearrange("p (s t) -> p s t", s=3)
            # planes 0,1 fused: (yA,zB)*512 + (px,py)
            nc.vector.scalar_tensor_tensor(
                out=idx16[:, 0:2 * NIC], in0=ci[:, 2 * NIC:4 * NIC], scalar=RES,
                in1=ci[:, 0:2 * NIC], op0=mult, op1=add)
            nc.vector.scalar_tensor_tensor(
                out=iv16[:, 2], in0=civ[:, 3], scalar=RES, in1=civ[:, 0],
                op0=mult, op1=add)

            # ---- gathers: one dma_gather per plane from its band slice
            # SWDGE ring caps one DMA at <128 descs/engine (NI/16+1), so
            # split each plane's 2048-row gather into 2x1024, spread round-
            # robin over the 4 SWDGE queues (parallel Q7 pairs).
            gt = gpool.tile([128, 3, SJ, 128], tdt, tag="g", name="g")
            for pl in range(3):
                base = BCELLS + pl * CELLS + (ab if pl == 0 else bb) * BCELLS
                half = S // 2
                for hh in range(2):
                    nc.gpsimd.dma_gather(
                        gt[:, pl, hh * (SJ // 2):(hh + 1) * (SJ // 2), :],
                        ptd[base:base + BCELLS, :],
                        idx16[:, pl * NIC + hh * (NIC // 2):
                              pl * NIC + (hh + 1) * (NIC // 2)],
                        half, half, 128,
                        queue_num=(g * 6 + pl * 2 + hh) % 4)

            # ---- bilinear weights (p-major layout: point i at (i%128, i//128))
            # cw streams (pl-major): [y(pl0) y(pl1) y(pl2) x(pl0) x(pl1) x(pl2)]
            cw = wt([128, 6 * SJ], f32, "cw")
            nc.sync.dma_start(cw[:], crdw[g])
            pix = wt([128, 6 * SJ], f32, "pix")
            nc.scalar.activation(pix[:], cw[:], AF.Copy, bias=255.5, scale=255.5)
            pixm = wt([128, 6 * SJ], f32, "pixm")
            nc.scalar.activation(pixm[:], cw[:], AF.Copy, bias=_cast_bias(),
                                 scale=255.5)
            ciw = wt([128, 6 * SJ], i32, "ciw")
            nc.scalar.activation(ciw[:], pixm[:], AF.Copy)
            cfw = wt([128, 6 * SJ], f32, "cfw")
            nc.scalar.activation(cfw[:], ciw[:], AF.Copy)
            # frp[p, s, j, k]: k=0 -> 1-frac, k=1 -> frac  (s = 6 streams)
            frp = wt([128, 6 * SJ * 2], bf16, "frp")
            frv = frp[:].rearrange("p (s j k) -> p s j k", s=6, k=2)
            nc.vector.tensor_sub(
                frv[:, :, :, 1],
                pix[:].rearrange("p (s j) -> p s j", s=6),
                cfw[:].rearrange("p (s j) -> p s j", s=6))
            nc.vector.tensor_scalar(out=frv[:, :, :, 0], in0=frv[:, :, :, 1],
                                    scalar1=-1.0, scalar2=1.0,
                                    op0=mult, op1=add)
            # corner weights: W[p, pl, j, ky, kx] = wy[ky] * wx[kx]; corner
            # order matches PT rows [(y,x),(y,x+1),(y+1,x),(y+1,x+1)]
            wts4 = wt([128, 3 * SJ * 4], bf16, "wts4")
            wv5 = wts4[:].rearrange("p (pl j a b) -> p pl j a b", pl=3, a=2, b=2)
            nc.vector.tensor_tensor(
                out=wv5,
                in0=frv[:, 0:3].unsqueeze(4).to_broadcast([128, 3, SJ, 2, 2]),
                in1=frv[:, 3:6].unsqueeze(3).to_broadcast([128, 3, SJ, 2, 2]),
                op=mult)
            w4v = wts4[:].rearrange("p (pl j c) -> p pl j c", pl=3, c=4)

            # ---- combine: one big prod -> corner-pair tree -> plane sum
            prod = wt([128, 3 * SJ * 4 * EMB], bf16, "prod")
            nc.vector.tensor_tensor(
                out=prod[:].rearrange("p (a c e) -> p a c e", c=4, e=EMB),
                in0=gt[:].rearrange("p pl j (c e) -> p (pl j) c e", c=4),
                in1=w4v[:].rearrange("p pl j c -> p (pl j) c").unsqueeze(
                    3).to_broadcast([128, 3 * SJ, 4, EMB]),
                op=mult)
            p5 = prod[:].rearrange("p (a h l e) -> p a h l e", h=2, l=2, e=EMB)
            s2 = wt([128, 3 * SJ * 2 * EMB], bf16, "s2")
            s2v = s2[:].rearrange("p (a h e) -> p a h e", h=2, e=EMB)
            nc.vector.tensor_tensor(out=s2v, in0=p5[:, :, :, 0],
                                    in1=p5[:, :, :, 1], op=add)
            s1 = wt([128, 3 * SJ * EMB], bf16, "s1")
            s1v = s1[:].rearrange("p (pl j e) -> p pl j e", pl=3, e=EMB)
            nc.vector.tensor_tensor(
                out=s1[:].rearrange("p (a e) -> p a e", e=EMB),
                in0=s2v[:, :, 0], in1=s2v[:, :, 1], op=add)
            feats = wt([128, SJ * EMB], bf16, "feats")
            fv = feats[:].rearrange("p (j e) -> p j e", e=EMB)
            nc.vector.tensor_tensor(out=fv, in0=s1v[:, 0], in1=s1v[:, 1], op=add)
            nc.vector.tensor_tensor(out=fv, in0=fv, in1=s1v[:, 2], op=add)

            # ---- MLP (batches of 512 points)
            for bt in range(NBT):
                ftp = psum.tile([EMB, 4 * 128], bf16, tag="ftp", name="ftp",
                                space="PSUM", bufs=2)
                for kk in range(4):
                    nc.tensor.transpose(
                        out=ftp[:, kk * 128:(kk + 1) * 128],
                        in_=feats[:, (bt * 4 + kk) * EMB:(bt * 4 + kk + 1) * EMB],
                        identity=ident[:])
                fts = wt([EMB, 4 * 128], bf16, "fts")
                nc.scalar.activation(fts[:], ftp[:], AF.Copy)

                mm0 = psum.tile([HID, 4 * 128], f32, tag="mm", name="mm",
                                space="PSUM", bufs=3)
                nc.tensor.matmul(out=mm0[:], lhsT=w0s[:], rhs=fts[:],
                                 start=True, stop=True)
                h0 = wt([HID, 4 * 128], bf16, "h0")
                nc.scalar.activation(h0[:], mm0[:], AF.Relu, bias=b0s[:, 0:1])

                mm1 = psum.tile([HID, 4 * 128], f32, tag="mm", name="mm",
                                space="PSUM", bufs=3)
                nc.tensor.matmul(out=mm1[:], lhsT=w1s[:], rhs=h0[:],
                                 start=True, stop=True)
                h1 = wt([HID, 4 * 128], bf16, "h1")
                nc.vector.tensor_scalar(out=h1[:], in0=mm1[:],
                                        scalar1=b1s[:, 0:1], scalar2=0.0,
                                        op0=add, op1=amax)

                mm2 = psum.tile([HID, 4 * 128], f32, tag="mm", name="mm",
                                space="PSUM", bufs=3)
                nc.tensor.matmul(out=mm2[:], lhsT=w2s[:], rhs=h1[:],
                                 start=True, stop=True)
                h2 = wt([HID, 4 * 128], bf16, "h2")
                nc.scalar.activation(h2[:], mm2[:], AF.Relu, bias=b2s[:, 0:1])

                mm3 = psum.tile([1, 4 * 128], f32, tag="mm3", name="mm3",
                                space="PSUM", bufs=2)
                nc.tensor.matmul(out=mm3[:], lhsT=w3s[:], rhs=h2[:],
                                 start=True, stop=True)
                res = wt([1, 4 * 128], f32, "res")
                nc.scalar.activation(res[:], mm3[:], AF.Identity,
                                     bias=b3s[0:1, 0:1])
                nc.sync.dma_start(
                    outv[:, g * S + bt * 512:g * S + (bt + 1) * 512], res[:])

    nc.finalize()
    return nc


def _get_nc():
    key = ("bfloat16", SIM_MODE)
    if key not in _BUILT:
        _BUILT[key] = _build_nc("bfloat16")
    return _BUILT[key]


def _build_patch_table(planes: np.ndarray, np_dt) -> np.ndarray:
    # planes [3, 32, 512, 512] -> PT [3*512*512, 128], +BCELLS guard rows
    # on each end (see the dram_tensor comment in _build_nc)
    p = planes.transpose(0, 2, 3, 1)  # [3, H, W, C]
    pt = np.zeros((3, RES, RES, 4, EMB), dtype=np.float32)
    pt[:, :, :, 0] = p
    pt[:, :, :-1, 1] = p[:, :, 1:]
    pt[:, :-1, :, 2] = p[:, 1:]
    pt[:, :-1, :-1, 3] = p[:, 1:, 1:]
    full = np.zeros((3 * CELLS + 2 * BCELLS, 4 * EMB), np.float32)
    full[BCELLS:BCELLS + 3 * CELLS] = pt.reshape(3 * CELLS, 4 * EMB)
    return np.ascontiguousarray(full).astype(np_dt)


def _floor_pix(c: np.ndarray) -> np.ndarray:
    # must match device: ACT (c*255.5 + bias) then int32 cast
    pm = c.astype(np.float32) * np.float32(255.5) + np.float32(_cast_bias())
    if SIM_MODE:
        return pm.astype(np.int32)  # numpy trunc, like CoreSim
    return np.rint(pm).astype(np.int32)  # HW rint


def kernel(**inputs: np.ndarray) -> np.ndarray:
    global LAST_RESULTS
    import ml_dtypes
    from concourse.bass_utils import run_bass_kernel_spmd

    coords = np.asarray(inputs["coordinates"], dtype=np.float32)
    planes = np.asarray(inputs["planes"], dtype=np.float32)
    bf = ml_dtypes.bfloat16
    pt = _build_patch_table(planes, bf)
    w0t = np.ascontiguousarray(inputs["w0"].T).astype(bf)
    w1t = np.ascontiguousarray(inputs["w1"].T).astype(bf)
    w2t = np.ascontiguousarray(inputs["w2"].T).astype(bf)
    w3t = np.ascontiguousarray(inputs["w3"].T).astype(bf)
    b0 = np.asarray(inputs["b0"], np.float32).reshape(HID, 1)
    b1 = np.asarray(inputs["b1"], np.float32).reshape(HID, 1)
    b2 = np.asarray(inputs["b2"], np.float32).reshape(1 * HID, 1)
    b3 = np.asarray(inputs["b3"], np.float32).reshape(1, 1)

    n = coords.shape[0]
    y0 = _floor_pix(coords[:, 1])
    z0 = _floor_pix(coords[:, 2])
    ab = np.clip(y0 >> 6, 0, BANDS - 1)
    bb = np.clip(z0 >> 6, 0, BANDS - 1)
    bucket = (ab * BANDS + bb).astype(np.int64)

    order = np.argsort(bucket, kind="stable")
    sorted_ids = order
    counts = np.bincount(bucket, minlength=NGRP)
    starts = np.concatenate(([0], np.cumsum(counts)))

    # slot_map[core, g, i] = original point id (or -1 for pad)
    slot_map = np.full((NCORES, NGRP, S), -1, dtype=np.int64)
    crdw = np.zeros((NCORES, NGRP, 128, 6 * SJ), np.float32)
    crdi = np.zeros((NCORES, NGRP, 128, 3 * NIC), np.float32)
    # device weight streams: y-coord per plane then x-coord per plane
    cmap = np.array([1, 2, 2, 0, 1, 0])

    for g in range(NGRP):
        ids = sorted_ids[starts[g]:starts[g + 1]]
        nk = len(ids)
        assert nk <= NCORES * S, f"bucket {g} overflow: {nk}"
        for c in range(NCORES):
            chunk = ids[c::NCORES]
            m = len(chunk)
            assert m <= S, f"core chunk overflow: {m}"
            slot_map[c, g, :m] = chunk
            if m == 0:
                # fabricate an in-band point (weights harmless, output unused)
                aa, zz = g // BANDS, g % BANDS
                fake = np.array([0.0,
                                 (64 * aa + 32) / 255.5 - 1.0,
                                 (64 * zz + 32) / 255.5 - 1.0], np.float32)
                pts = np.tile(fake, (S, 1))
            else:
                pts = coords[chunk]
                if m < S:
                    pts = np.concatenate(
                        [pts, np.tile(pts[0], (S - m, 1))], axis=0)
            # [p, s, j]: coord cmap[s] of point p + 128j
            pw = pts.reshape(SJ, 128, 3)[:, :, cmap]  # [j, p, s]
            crdw[c, g] = pw.transpose(1, 2, 0).reshape(128, 6 * SJ)
            arr = pts.reshape(NIC, 16, 3).transpose(1, 2, 0).reshape(
                16, 3 * NIC)  # [q, (c, t)]
            crdi[c, g] = np.tile(arr, (8, 1))

    in_maps = []
    for c in range(NCORES):
        in_maps.append({
            "pt": pt,
            "crdw": np.ascontiguousarray(crdw[c]),
            "crdi": np.ascontiguousarray(crdi[c]),
            "w0t": w0t, "w1t": w1t, "w2t": w2t, "w3t": w3t,
            "b0c": b0, "b1c": b1, "b2c": b2, "b3c": b3,
        })

    nc = _get_nc()
    LAST_RESULTS = run_bass_kernel_spmd(nc, in_maps, list(range(NCORES)))

    full = np.zeros(n, np.float32)
    for c in range(NCORES):
        o = np.asarray(LAST_RESULTS.results[c]["out"], np.float32).ravel()
        sm = slot_map[c].ravel()
        valid = sm >= 0
        full[sm[valid]] = o[valid]
    return full.reshape(1, n, 1).astype(np.float32)


# revision 25
# speedup vs baseline: 1.7286x; 1.7286x over previous
"""Triplane embedding-lookup + MLP kernel for Trainium2 (8 NeuronCores).

Strategy (v2, dma_gather):
  - Host: patch table PT[3*512*512, 128] bf16; row (pl,y,x) = 4 bilinear-corner
    pixel vectors (32ch each). Points are bucketed by (y-band, z-band) into
    8x8=64 groups of 2048 slots per core, so every group's gathers hit a
    <=32768-row band slice of PT -- addressable by dma_gather's int16 indices.
  - Device, per group: compute local cell ids (int16) + bilinear weights,
    3 dma_gather calls (2048 rows each, one SWDGE instr apiece -- ~20x less
    Pool time than per-partition indirect DMA), bf16 weighted combine on DVE,
    4-layer MLP on PE (bf16), out [1, 2048] f32 to DRAM.
  - Host: inverse-permute the per-slot outputs back to input point order.
"""

import sys

sys.path.insert(0, "/opt/trn_rl_repo")

from contextlib import ExitStack

import numpy as np

RES = 512
CELLS = RES * RES
EMB = 32
HID = 128
N = 1_000_000
NCORES = 8

BANDS = 8          # bands per axis
BROWS = RES // BANDS   # 64 rows per band
BCELLS = BROWS * RES   # 32768 cells per band slice (int16-addressable)
NGRP = BANDS * BANDS   # 64 groups per core: (y-band, z-band)
S = 2048           # point slots per group
SJ = S // 128      # 16 free-dim cols per partition (weights layout)
NIC = S // 16      # 128 idx cols per partition (idx layout)
NBT = 4            # MLP batches of 512 points per group
NP_CORE = NGRP * S  # 131072 slots per core

LAST_RESULTS = None
_BUILT = {}

# HW f32->i32 cast is rint => floor(pix) == rint(pix - 0.5), bias 255.0.
# CoreSim casts via numpy truncation => floor(pix) == trunc(pix), bias 255.5.
SIM_MODE = False


def _cast_bias() -> float:
    return 255.5 if SIM_MODE else 255.0


def _build_nc(table_dt_name: str = "bfloat16"):
    from concourse import bacc, bass, mybir, library_config
    import concourse.tile as tile
    from concourse.masks import make_identity

    dt = mybir.dt
    tdt = getattr(dt, table_dt_name)
    f32 = dt.float32
    i32 = dt.int32
    i16 = dt.int16
    bf16 = dt.bfloat16
    mult = mybir.AluOpType.mult
    add = mybir.AluOpType.add
    amax = mybir.AluOpType.max
    amin = mybir.AluOpType.min
    AF = mybir.ActivationFunctionType

    # 4 SWDGE queues => 4 Q7 core-pairs generate gather descriptors
    # concurrently (measured 3.6x on HW vs 1 queue).
    nc = bacc.Bacc("TRN2", target_bir_lowering=False, num_swdge_queues=4)

    # BCELLS guard rows on both ends: wrapped/mismatched int16 indices
    # (+-32K rows) always land inside the tensor instead of faulting.
    ptd = nc.dram_tensor("pt", [3 * CELLS + 2 * BCELLS, 128], tdt,
                         kind="ExternalInput")
    crdw = nc.dram_tensor("crdw", [NGRP, 128, 6 * SJ], f32, kind="ExternalInput")
    crdi = nc.dram_tensor("crdi", [NGRP, 128, 3 * NIC], f32, kind="ExternalInput")
    w0d = nc.dram_tensor("w0t", [EMB, HID], bf16, kind="ExternalInput")
    w1d = nc.dram_tensor("w1t", [HID, HID], bf16, kind="ExternalInput")
    w2d = nc.dram_tensor("w2t", [HID, HID], bf16, kind="ExternalInput")
    w3d = nc.dram_tensor("w3t", [HID, 1], bf16, kind="ExternalInput")
    b0d = nc.dram_tensor("b0c", [HID, 1], f32, kind="ExternalInput")
    b1d = nc.dram_tensor("b1c", [HID, 1], f32, kind="ExternalInput")
    b2d = nc.dram_tensor("b2c", [HID, 1], f32, kind="ExternalInput")
    b3d = nc.dram_tensor("b3c", [1, 1], f32, kind="ExternalInput")
    outd = nc.dram_tensor("out", [NP_CORE], f32, kind="ExternalOutput")
    outv = outd[:].unsqueeze(0)

    with tile.TileContext(nc) as tc, ExitStack() as ctx:
        nc.gpsimd.load_library(library_config.mlp)

        cpool = ctx.enter_context(tc.tile_pool(name="consts", bufs=1))

        def const_tile(shape, dtp, tag):
            return cpool.tile(shape, dtp, tag=tag, name=tag)

        w0s = const_tile([EMB, HID], bf16, "w0s")
        w1s = const_tile([HID, HID], bf16, "w1s")
        w2s = const_tile([HID, HID], bf16, "w2s")
        w3s = const_tile([HID, 1], bf16, "w3s")
        b0s = const_tile([HID, 1], f32, "b0s")
        b1s = const_tile([HID, 1], f32, "b1s")
        b2s = const_tile([HID, 1], f32, "b2s")
        b3s = const_tile([1, 1], f32, "b3s")
        ident = const_tile([128, 128], bf16, "ident")
        for s_, d_ in ((w0s, w0d), (w1s, w1d), (w2s, w2d), (w3s, w3d),
                       (b0s, b0d), (b1s, b1d), (b2s, b2d), (b3s, b3d)):
            nc.sync.dma_start(s_[:], d_[:])
        make_identity(nc, ident[:])

        work = ctx.enter_context(tc.tile_pool(name="work", bufs=2))
        gpool = ctx.enter_context(tc.tile_pool(name="gather", bufs=2))
        psum = ctx.enter_context(tc.tile_pool(name="psum", bufs=2, space="PSUM"))

        def wt(shape, dtp, tag, bufs=2):
            return work.tile(shape, dtp, tag=tag, name=tag, bufs=bufs)

        for g in range(NGRP):
            ab, bb = g // BANDS, g % BANDS

            # ---- index path (idx-wrapped layout: point i at (i%16, i//16),
            #      replicated x8 down partitions; cols [x NIC | y NIC | z NIC])
            ct = wt([128, 3 * NIC], f32, "ct", bufs=3)
            nc.sync.dma_start(ct[:], crdi[g])
            pm = wt([128, 4 * NIC], f32, "pm", bufs=3)
            cb = _cast_bias()
            nc.scalar.activation(pm[:, 0:2 * NIC], ct[:, 0:2 * NIC], AF.Copy,
                                 bias=cb, scale=255.5)              # px|py
            nc.scalar.activation(pm[:, 2 * NIC:3 * NIC], ct[:, NIC:2 * NIC],
                                 AF.Copy, bias=cb - 64.0 * ab, scale=255.5)
            nc.scalar.activation(pm[:, 3 * NIC:4 * NIC], ct[:, 2 * NIC:3 * NIC],
                                 AF.Copy, bias=cb - 64.0 * bb, scale=255.5)
            ci = wt([128, 4 * NIC], i32, "ci", bufs=3)
            nc.scalar.activation(ci[:], pm[:], AF.Copy)  # f32->i32 rint on HW
            civ = ci[:].rearrange("p (s t) -> p s t", s=4)
            # int16 out directly (values <= 32766 when host/device agree;
            # rare mismatches wrap into the table's guard rows)
            idx16 = wt([128, 3 * NIC], i16, "idx16", bufs=3)
            iv16 = idx16[:].rearrange("p (s t) -> p s t", s=3)
            # planes 0,1 fused: (yA,zB)*512 + (px,py)
            nc.vector.scalar_tensor_tensor(
                out=idx16[:, 0:2 * NIC], in0=ci[:, 2 * NIC:4 * NIC], scalar=RES,
                in1=ci[:, 0:2 * NIC], op0=mult, op1=add)
            nc.vector.scalar_tensor_tensor(
                out=iv16[:, 2], in0=civ[:, 3], scalar=RES, in1=civ[:, 0],
                op0=mult, op1=add)

            # ---- gathers: one dma_gather per plane from its band slice
            # SWDGE ring caps one DMA at <128 descs/engine (NI/16+1), so
            # split each plane's 2048-row gather into 2x1024, spread round-
            # robin over the 4 SWDGE queues (parallel Q7 pairs).
            gts = []
            for pl in range(3):
                base = BCELLS + pl * CELLS + (ab if pl == 0 else bb) * BCELLS
                gt = gpool.tile([128, SJ, 128], tdt, tag=f"g{pl}", name=f"g{pl}")
                half = S // 2
                for hh in range(2):
                    nc.gpsimd.dma_gather(
                        gt[:, hh * (SJ // 2):(hh + 1) * (SJ // 2), :],
                        ptd[base:base + BCELLS, :],
                        idx16[:, pl * NIC + hh * (NIC // 2):
                              pl * NIC + (hh + 1) * (NIC // 2)],
                        half, half, 128,
                        queue_num=(g * 6 + pl * 2 + hh) % 4)
                gts.append(gt)

            # ---- bilinear weights (p-major layout: point i at (i%128, i//128))
            # cw streams (pl-major): [y(pl0) y(pl1) y(pl2) x(pl0) x(pl1) x(pl2)]
            cw = wt([128, 6 * SJ], f32, "cw")
            nc.sync.dma_start(cw[:], crdw[g])
            pix = wt([128, 6 * SJ], f32, "pix")
            nc.scalar.activation(pix[:], cw[:], AF.Copy, bias=255.5, scale=255.5)
            pixm = wt([128, 6 * SJ], f32, "pixm")
            nc.scalar.activation(pixm[:], cw[:], AF.Copy, bias=_cast_bias(),
                                 scale=255.5)
            ciw = wt([128, 6 * SJ], i32, "ciw")
            nc.scalar.activation(ciw[:], pixm[:], AF.Copy)
            cfw = wt([128, 6 * SJ], f32, "cfw")
            nc.scalar.activation(cfw[:], ciw[:], AF.Copy)
            # frp[p, s, j, k]: k=0 -> 1-frac, k=1 -> frac  (s = 6 streams)
            frp = wt([128, 6 * SJ * 2], bf16, "frp")
            frv = frp[:].rearrange("p (s j k) -> p s j k", s=6, k=2)
            nc.vector.tensor_sub(
                frv[:, :, :, 1],
                pix[:].rearrange("p (s j) -> p s j", s=6),
                cfw[:].rearrange("p (s j) -> p s j", s=6))
            nc.vector.tensor_scalar(out=frv[:, :, :, 0], in0=frv[:, :, :, 1],
                                    scalar1=-1.0, scalar2=1.0,
                                    op0=mult, op1=add)
            # corner weights: W[p, pl, j, ky, kx] = wy[ky] * wx[kx]; corner
            # order matches PT rows [(y,x),(y,x+1),(y+1,x),(y+1,x+1)]
            wts4 = wt([128, 3 * SJ * 4], bf16, "wts4")
            wv5 = wts4[:].rearrange("p (pl j a b) -> p pl j a b", pl=3, a=2, b=2)
            nc.vector.tensor_tensor(
                out=wv5,
                in0=frv[:, 0:3].unsqueeze(4).to_broadcast([128, 3, SJ, 2, 2]),
                in1=frv[:, 3:6].unsqueeze(3).to_broadcast([128, 3, SJ, 2, 2]),
                op=mult)
            w4v = wts4[:].rearrange("p (pl j c) -> p pl j c", pl=3, c=4)

            # ---- combine: per plane prod -> corner-pair tree -> plane sum
            s1 = wt([128, 3 * SJ * EMB], bf16, "s1")
            s1v = s1[:].rearrange("p (pl j e) -> p pl j e", pl=3, e=EMB)
            for pl in range(3):
                prod = wt([128, SJ * 4 * EMB], bf16, f"prod{pl}")
                pv = prod[:].rearrange("p (j c e) -> p j c e", c=4, e=EMB)
                g4 = gts[pl][:].rearrange("p j (c e) -> p j c e", c=4)
                wb = w4v[:, pl].unsqueeze(3).to_broadcast([128, SJ, 4, EMB])
                nc.vector.tensor_tensor(out=pv, in0=g4, in1=wb, op=mult)
                p5 = prod[:].rearrange("p (j h l e) -> p j h l e", h=2, l=2, e=EMB)
                s2 = wt([128, SJ * 2 * EMB], bf16, f"s2_{pl}")
                s2v = s2[:].rearrange("p (j h e) -> p j h e", h=2, e=EMB)
                nc.vector.tensor_tensor(out=s2v, in0=p5[:, :, :, 0],
                                        in1=p5[:, :, :, 1], op=add)
                nc.vector.tensor_tensor(out=s1v[:, pl], in0=s2v[:, :, 0],
                                        in1=s2v[:, :, 1], op=add)
            feats = wt([128, SJ * EMB], bf16, "feats")
            fv = feats[:].rearrange("p (j e) -> p j e", e=EMB)
            nc.vector.tensor_tensor(out=fv, in0=s1v[:, 0], in1=s1v[:, 1], op=add)
            nc.vector.tensor_tensor(out=fv, in0=fv, in1=s1v[:, 2], op=add)

            # ---- MLP (batches of 512 points)
            for bt in range(NBT):
                ftp = psum.tile([EMB, 4 * 128], bf16, tag="ftp", name="ftp",
                                space="PSUM", bufs=2)
                for kk in range(4):
                    nc.tensor.transpose(
                        out=ftp[:, kk * 128:(kk + 1) * 128],
                        in_=feats[:, (bt * 4 + kk) * EMB:(bt * 4 + kk + 1) * EMB],
                        identity=ident[:])
                fts = wt([EMB, 4 * 128], bf16, "fts")
                nc.scalar.activation(fts[:], ftp[:], AF.Copy)

                mm0 = psum.tile([HID, 4 * 128], f32, tag="mm", name="mm",
                                space="PSUM", bufs=3)
                nc.tensor.matmul(out=mm0[:], lhsT=w0s[:], rhs=fts[:],
                                 start=True, stop=True)
                h0 = wt([HID, 4 * 128], bf16, "h0")
                nc.scalar.activation(h0[:], mm0[:], AF.Relu, bias=b0s[:, 0:1])

                mm1 = psum.tile([HID, 4 * 128], f32, tag="mm", name="mm",
                                space="PSUM", bufs=3)
                nc.tensor.matmul(out=mm1[:], lhsT=w1s[:], rhs=h0[:],
                                 start=True, stop=True)
                h1 = wt([HID, 4 * 128], bf16, "h1")
                nc.vector.tensor_scalar(out=h1[:], in0=mm1[:],
                                        scalar1=b1s[:, 0:1], scalar2=0.0,
                                        op0=add, op1=amax)

                mm2 = psum.tile([HID, 4 * 128], f32, tag="mm", name="mm",
                                space="PSUM", bufs=3)
                nc.tensor.matmul(out=mm2[:], lhsT=w2s[:], rhs=h1[:],
                                 start=True, stop=True)
                h2 = wt([HID, 4 * 128], bf16, "h2")
                nc.scalar.activation(h2[:], mm2[:], AF.Relu, bias=b2s[:, 0:1])

                mm3 = psum.tile([1, 4 * 128], f32, tag="mm3", name="mm3",
                                space="PSUM", bufs=2)
                nc.tensor.matmul(out=mm3[:], lhsT=w3s[:], rhs=h2[:],
                                 start=True, stop=True)
                res = wt([1, 4 * 128], f32, "res")
                nc.scalar.activation(res[:], mm3[:], AF.Identity,
                                     bias=b3s[0:1, 0:1])
                nc.sync.dma_start(
                    outv[:, g * S + bt * 512:g * S + (bt + 1) * 512], res[:])

    nc.finalize()
    return nc


def _get_nc():
    key = ("bfloat16", SIM_MODE)
    if key not in _BUILT:
        _BUILT[key] = _build_nc("bfloat16")
    return _BUILT[key]


def _build_patch_table(planes: np.ndarray, np_dt) -> np.ndarray:
    # planes [3, 32, 512, 512] -> PT [3*512*512, 128], +BCELLS guard rows
    # on each end (see the dram_tensor comment in _build_nc)
    p = planes.transpose(0, 2, 3, 1)  # [3, H, W, C]
    pt = np.zeros((3, RES, RES, 4, EMB), dtype=np.float32)
    pt[:, :, :, 0] = p
    pt[:, :, :-1, 1] = p[:, :, 1:]
    pt[:, :-1, :, 2] = p[:, 1:]
    pt[:, :-1, :-1, 3] = p[:, 1:, 1:]
    full = np.zeros((3 * CELLS + 2 * BCELLS, 4 * EMB), np.float32)
    full[BCELLS:BCELLS + 3 * CELLS] = pt.reshape(3 * CELLS, 4 * EMB)
    return np.ascontiguousarray(full).astype(np_dt)


def _floor_pix(c: np.ndarray) -> np.ndarray:
    # must match device: ACT (c*255.5 + bias) then int32 cast
    pm = c.astype(np.float32) * np.float32(255.5) + np.float32(_cast_bias())
    if SIM_MODE:
        return pm.astype(np.int32)  # numpy trunc, like CoreSim
    return np.rint(pm).astype(np.int32)  # HW rint


def kernel(**inputs: np.ndarray) -> np.ndarray:
    global LAST_RESULTS
    import ml_dtypes
    from concourse.bass_utils import run_bass_kernel_spmd

    coords = np.asarray(inputs["coordinates"], dtype=np.float32)
    planes = np.asarray(inputs["planes"], dtype=np.float32)
    bf = ml_dtypes.bfloat16
    pt = _build_patch_table(planes, bf)
    w0t = np.ascontiguousarray(inputs["w0"].T).astype(bf)
    w1t = np.ascontiguousarray(inputs["w1"].T).astype(bf)
    w2t = np.ascontiguousarray(inputs["w2"].T).astype(bf)
    w3t = np.ascontiguousarray(inputs["w3"].T).astype(bf)
    b0 = np.asarray(inputs["b0"], np.float32).reshape(HID, 1)
    b1 = np.asarray(inputs["b1"], np.float32).reshape(HID, 1)
    b2 = np.asarray(inputs["b2"], np.float32).reshape(1 * HID, 1)
    b3 = np.asarray(inputs["b3"], np.float32).reshape(1, 1)

    n = coords.shape[0]
    y0 = _floor_pix(coords[:, 1])
    z0 = _floor_pix(coords[:, 2])
    ab = np.clip(y0 >> 6, 0, BANDS - 1)
    bb = np.clip(z0 >> 6, 0, BANDS - 1)
    bucket = (ab * BANDS + bb).astype(np.int64)

    order = np.argsort(bucket, kind="stable")
    sorted_ids = order
    counts = np.bincount(bucket, minlength=NGRP)
    starts = np.concatenate(([0], np.cumsum(counts)))

    # slot_map[core, g, i] = original point id (or -1 for pad)
    slot_map = np.full((NCORES, NGRP, S), -1, dtype=np.int64)
    crdw = np.zeros((NCORES, NGRP, 128, 6 * SJ), np.float32)
    crdi = np.zeros((NCORES, NGRP, 128, 3 * NIC), np.float32)
    # device weight streams: y-coord per plane then x-coord per plane
    cmap = np.array([1, 2, 2, 0, 1, 0])

    for g in range(NGRP):
        ids = sorted_ids[starts[g]:starts[g + 1]]
        nk = len(ids)
        assert nk <= NCORES * S, f"bucket {g} overflow: {nk}"
        for c in range(NCORES):
            chunk = ids[c::NCORES]
            m = len(chunk)
            assert m <= S, f"core chunk overflow: {m}"
            slot_map[c, g, :m] = chunk
            if m == 0:
                # fabricate an in-band point (weights harmless, output unused)
                aa, zz = g // BANDS, g % BANDS
                fake = np.array([0.0,
                                 (64 * aa + 32) / 255.5 - 1.0,
                                 (64 * zz + 32) / 255.5 - 1.0], np.float32)
                pts = np.tile(fake, (S, 1))
            else:
                pts = coords[chunk]
                if m < S:
                    pts = np.concatenate(
                        [pts, np.tile(pts[0], (S - m, 1))], axis=0)
            # [p, s, j]: coord cmap[s] of point p + 128j
            pw = pts.reshape(SJ, 128, 3)[:, :, cmap]  # [j, p, s]
            crdw[c, g] = pw.transpose(1, 2, 0).reshape(128, 6 * SJ)
            arr = pts.reshape(NIC, 16, 3).transpose(1, 2, 0).reshape(
                16, 3 * NIC)  # [q, (c, t)]
            crdi[c, g] = np.tile(arr, (8, 1))

    in_maps = []
    for c in range(NCORES):
        in_maps.append({
            "pt": pt,
            "crdw": np.ascontiguousarray(crdw[c]),
            "crdi": np.ascontiguousarray(crdi[c]),
            "w0t": w0t, "w1t": w1t, "w2t": w2t, "w3t": w3t,
            "b0c": b0, "b1c": b1, "b2c": b2, "b3c": b3,
        })

    nc = _get_nc()
    LAST_RESULTS = run_bass_kernel_spmd(nc, in_maps, list(range(NCORES)))

    full = np.zeros(n, np.float32)
    for c in range(NCORES):
        o = np.asarray(LAST_RESULTS.results[c]["out"], np.float32).ravel()
        sm = slot_map[c].ravel()
        valid = sm >= 0
        full[sm[valid]] = o[valid]
    return full.reshape(1, n, 1).astype(np.float32)
